# revision 1
# baseline (speedup 1.0000x reference)
"""Triangular GEMM C = triu(triu(A) @ triu(B)) for N=4096 fp32 on 8 trn2 cores.

Block decomposition (T=512): C(I,J) = sum_{K=I..J} A(I,K) @ B(K,J) for I<=J,
with diagonal A/B blocks pre-masked triu on host. 120 unit block-matmuls.

Work is packed into a uniform SPMD program (one compiled kernel, per-core
behavior lives entirely in host-packed DRAM stacks):

  per core: 1 "brick" = two depth-4 PSUM K-chains sharing their 4 stationary
  blocks, + 3 groups of singles (sizes 3,2,2) sharing one moving block each.
  = 15 units, 22 input blocks (22 MB), 9 output partials (9 MB).

Transpose trick: C = tA@tB  <=>  C^T = tB^T @ tA^T, so a column-sharing
(B-side) brick/group runs the same program with A/B roles swapped in the
host packing and its output partial transposed on unpack. Uniformity is
preserved; the mode is invisible to the device program.

Host scatter-adds the per-core partials into C. Entries below the diagonal
are exactly zero (every product has a zero factor), matching the reference.
"""

import numpy as np

N = 4096
T = 512  # block size
NB = N // T  # 8
P = 128
KSUB = T // P  # 4
NCORES = 8
NSLOTS = 9

# float16 (e5m10) has the same 11-bit mantissa as float32r (TF32-like), so
# GEMM error is ~1.5e-4 either way (fp32 PSUM accumulation) -- but fp16
# halves input DMA traffic and keeps the fast weight-load path.
INPUT_DTYPE = "float16"  # float16 | float32r | float32
OUT_DTYPE = "float32"    # fp16 out saves no time (out-DMA fully hidden); keep accuracy
BUFS = dict(stat=4, mov=8, sh=3, ex=7, o=3, psum=8)  # full input residency
LOOP_KW = {}  # extra kwargs for the timing-only For_i repeat loop
PRELOAD = True   # issue all input DMAs up front
OUT_ENGINE = "gpsimd"  # separate DMA queue so stores never block input loads
COPY_ENGINE = "vector"  # psum->sbuf copy engine: any | vector | scalar

# 8 bricks: two 4-chains sharing the stationary panel.
#   N-mode: row I, stationary A(I,s) for s in S; chains produce C(I,J).
#   T-mode: col J, stationary B(s,J); chains produce C(I,J)^T.
BRICKS = [
    dict(mode="N", line=0, S=[0, 1, 2, 3], outs=[(0, 7), (0, 3)]),
    dict(mode="T", line=7, S=[4, 5, 6, 7], outs=[(0, 7), (4, 7)]),
    dict(mode="N", line=0, S=[0, 1, 2, 3], outs=[(0, 4), (0, 5)]),
    dict(mode="T", line=6, S=[1, 2, 3, 4], outs=[(0, 6), (1, 6)]),
    dict(mode="T", line=7, S=[2, 3, 4, 5], outs=[(1, 7), (2, 7)]),
    dict(mode="N", line=1, S=[1, 2, 3, 4], outs=[(1, 4), (1, 5)]),
    dict(mode="N", line=2, S=[2, 3, 4, 5], outs=[(2, 5), (2, 6)]),
    dict(mode="N", line=3, S=[3, 4, 5, 6], outs=[(3, 6), (3, 7)]),
]

# Singles groups (exact cover of the 56 leftover units; solver output).
# ('A', (i,k), units): shared A(i,k) -> trans mode;
# ('B', (k,j), units): shared B(k,j) -> normal mode. unit = (i,k,j).
GROUPS3 = [
    ("A", (4, 4), [(4, 4, 4), (4, 4, 5), (4, 4, 6)]),
    ("B", (4, 4), [(0, 4, 4), (2, 4, 4), (3, 4, 4)]),
    ("A", (5, 5), [(5, 5, 5), (5, 5, 6), (5, 5, 7)]),
    ("B", (5, 5), [(0, 5, 5), (1, 5, 5), (3, 5, 5)]),
    ("B", (6, 7), [(1, 6, 7), (2, 6, 7), (5, 6, 7)]),
    ("B", (6, 6), [(0, 6, 6), (1, 6, 6), (5, 6, 6)]),
    ("B", (7, 7), [(1, 7, 7), (2, 7, 7), (7, 7, 7)]),
    ("B", (7, 7), [(3, 7, 7), (5, 7, 7), (6, 7, 7)]),
]
GROUPS2 = [
    ("A", (0, 1), [(0, 1, 1), (0, 1, 2)]),
    ("A", (1, 2), [(1, 2, 2), (1, 2, 3)]),
    ("B", (2, 2), [(0, 2, 2), (2, 2, 2)]),
    ("A", (2, 2), [(2, 2, 3), (2, 2, 4)]),
    ("A", (2, 3), [(2, 3, 3), (2, 3, 4)]),
    ("B", (3, 3), [(1, 3, 3), (3, 3, 3)]),
    ("A", (3, 3), [(3, 3, 4), (3, 3, 5)]),
    ("B", (4, 5), [(0, 4, 5), (3, 4, 5)]),
    ("A", (4, 5), [(4, 5, 5), (4, 5, 6)]),
    ("B", (5, 6), [(0, 5, 6), (1, 5, 6)]),
    ("A", (0, 0), [(0, 0, 0), (0, 0, 1)]),
    ("A", (0, 0), [(0, 0, 2), (0, 0, 6)]),
    ("A", (1, 1), [(1, 1, 1), (1, 1, 2)]),
    ("A", (1, 1), [(1, 1, 3), (1, 1, 7)]),
    ("A", (6, 6), [(6, 6, 6), (6, 6, 7)]),
    ("B", (6, 6), [(2, 6, 6), (4, 6, 6)]),
]


def _core_schedule(c):
    """Packing directives for core c.

    Returns dict with block-spec lists; spec = (mat, bi, bj, pack) where
    mat in 'AB', pack 'L' (_pack_lhsT) or 'R' (_pack_rhs); and
    out_specs = [(I, J, transposed)] * 9.
    """
    br = BRICKS[c]
    stat, mov, out_specs = [], [], []
    if br["mode"] == "N":
        i = br["line"]
        stat = [("A", i, s, "L") for s in br["S"]]
        for (oi, oj) in br["outs"]:
            assert oi == i
            mov += [("B", s, oj, "R") for s in br["S"]]
            out_specs.append((oi, oj, False))
    else:
        j = br["line"]
        stat = [("B", s, j, "R") for s in br["S"]]
        for (oi, oj) in br["outs"]:
            assert oj == j
            mov += [("A", oi, s, "L") for s in br["S"]]
            out_specs.append((oi, oj, True))

    shared, excl = [], []
    for grp in [GROUPS3[c], GROUPS2[2 * c], GROUPS2[2 * c + 1]]:
        gmode, key, units = grp
        if gmode == "A":
            gi, gk = key
            shared.append(("A", gi, gk, "L"))
            for (ui, uk, uj) in units:
                assert (ui, uk) == key
                excl.append(("B", uk, uj, "R"))
                out_specs.append((ui, uj, True))
        else:
            gk, gj = key
            shared.append(("B", gk, gj, "R"))
            for (ui, uk, uj) in units:
                assert (uk, uj) == key
                excl.append(("A", ui, uk, "L"))
                out_specs.append((ui, uj, False))
    assert len(stat) == 4 and len(mov) == 8
    assert len(shared) == 3 and len(excl) == 7 and len(out_specs) == 9
    return dict(stat=stat, mov=mov, shared=shared, excl=excl, outs=out_specs)


_SCHEDULES = [_core_schedule(c) for c in range(NCORES)]


def _check_cover():
    seen = set()
    for c in range(NCORES):
        br = BRICKS[c]
        for (oi, oj) in br["outs"]:
            for s in br["S"]:
                u = (oi, s, oj) if br["mode"] == "N" else (oi, s, oj)
                assert oi <= s <= oj, (c, u)
                assert u not in seen, u
                seen.add(u)
        for grp in [GROUPS3[c], GROUPS2[2 * c], GROUPS2[2 * c + 1]]:
            for u in grp[2]:
                i, k, j = u
                assert i <= k <= j, u
                assert u not in seen, u
                seen.add(u)
    want = {(i, k, j) for i in range(NB) for k in range(i, NB)
            for j in range(k, NB)}
    assert seen == want, (len(seen), len(want))


_check_cover()

_PROGRAMS = {}


def _build_program(repeat=1):
    import contextlib
    import concourse.bacc as bacc
    import concourse.mybir as mybir
    from concourse.tile import TileContext

    dt_in = getattr(mybir.dt, INPUT_DTYPE)
    nc = bacc.Bacc("TRN2", target_bir_lowering=False, debug=False,
                   num_devices=NCORES)
    stat_in = nc.dram_tensor("stat4", [4, P, KSUB, T], dt_in,
                             kind="ExternalInput")
    mov_in = nc.dram_tensor("mov8", [8, P, KSUB, T], dt_in,
                            kind="ExternalInput")
    sh_in = nc.dram_tensor("shared3", [3, P, KSUB, T], dt_in,
                           kind="ExternalInput")
    ex_in = nc.dram_tensor("excl7", [7, P, KSUB, T], dt_in,
                           kind="ExternalInput")
    dt_out = getattr(mybir.dt, OUT_DTYPE)
    # [s, p, ms, n]: per-partition-contiguous rows -> full-rate DMA
    c_out = nc.dram_tensor("out_stack", [NSLOTS, P, KSUB, T],
                           dt_out, kind="ExternalOutput")

    f32 = mybir.dt.float32

    with TileContext(nc) as tc:
        with (
            tc.tile_pool(name="stat_pool", bufs=BUFS["stat"]) as stat_pool,
            tc.tile_pool(name="mov_pool", bufs=BUFS["mov"]) as mov_pool,
            tc.tile_pool(name="sh_pool", bufs=BUFS["sh"]) as sh_pool,
            tc.tile_pool(name="ex_pool", bufs=BUFS["ex"]) as ex_pool,
            tc.tile_pool(name="o_pool", bufs=BUFS["o"]) as o_pool,
            tc.tile_pool(name="psum", bufs=BUFS["psum"], space="PSUM") as psum_pool,
        ):
            out_eng = getattr(nc, OUT_ENGINE)
            copy_eng = getattr(nc, COPY_ENGINE)
            loop_ctx = (tc.For_i(0, repeat, 1, **LOOP_KW) if repeat > 1
                        else contextlib.nullcontext())
            with loop_ctx:
                def store(psums, slot):
                    o_t = o_pool.tile([P, KSUB, T], dt_out, tag="o",
                                      name=f"o_{slot}")
                    for ms in range(KSUB):
                        if COPY_ENGINE == "scalar":
                            copy_eng.copy(o_t[:, ms, :], psums[ms][:, :])
                        else:
                            copy_eng.tensor_copy(o_t[:, ms, :], psums[ms][:, :])
                    out_eng.dma_start(out=c_out[slot], in_=o_t)

                def load(pool, tag, name, src):
                    t_ = pool.tile([P, KSUB, T], dt_in, tag=tag, name=name)
                    nc.sync.dma_start(out=t_, in_=src)
                    return t_

                stat_t = [load(stat_pool, "st", f"st_{u}", stat_in[u])
                          for u in range(4)]
                if PRELOAD:
                    # issue in exact consumption order: brick movs, then
                    # each group's shared tile followed by its exclusives
                    mov_t = [load(mov_pool, "mv", f"mv_{u}", mov_in[u])
                             for u in range(8)]
                    sh_t_all, ex_t_all = [], []
                    e = 0
                    for g, gsize in enumerate([3, 2, 2]):
                        sh_t_all.append(load(sh_pool, "sh", f"sh_{g}",
                                             sh_in[g]))
                        for _ in range(gsize):
                            ex_t_all.append(load(ex_pool, "ex", f"ex_{e}",
                                                 ex_in[e]))
                            e += 1

                for ch in range(2):
                    psums = [psum_pool.tile([P, T], f32, tag="ps",
                                            name=f"ps_b{ch}_{m}")
                             for m in range(KSUB)]
                    for u in range(4):
                        m_t = (mov_t[ch * 4 + u] if PRELOAD else
                               load(mov_pool, "mv", f"mv_{ch}_{u}",
                                    mov_in[ch * 4 + u]))
                        for ks in range(KSUB):
                            rhs = m_t[:, ks, :]
                            for ms in range(KSUB):
                                nc.tensor.matmul(
                                    psums[ms][:, :],
                                    stat_t[u][:, ks, ms * P:(ms + 1) * P],
                                    rhs,
                                    start=(u == 0 and ks == 0),
                                    stop=(u == 3 and ks == KSUB - 1),
                                )
                    store(psums, ch)

                # singles groups (3, 2, 2)
                slot = 2
                e_idx = 0
                for g, gsize in enumerate([3, 2, 2]):
                    sh_t = (sh_t_all[g] if PRELOAD else
                            load(sh_pool, "sh", f"sh_{g}", sh_in[g]))
                    for q in range(gsize):
                        e_t = (ex_t_all[e_idx] if PRELOAD else
                               load(ex_pool, "ex", f"ex_{g}_{q}", ex_in[e_idx]))
                        psums = [psum_pool.tile([P, T], f32, tag="ps",
                                                name=f"ps_s{slot}_{m}")
                                 for m in range(KSUB)]
                        for ks in range(KSUB):
                            rhs = sh_t[:, ks, :]
                            for ms in range(KSUB):
                                nc.tensor.matmul(
                                    psums[ms][:, :],
                                    e_t[:, ks, ms * P:(ms + 1) * P],
                                    rhs,
                                    start=(ks == 0),
                                    stop=(ks == KSUB - 1),
                                )
                        store(psums, slot)
                        slot += 1
                        e_idx += 1
    nc.finalize()
    return nc


def _get_program(repeat=1):
    if repeat not in _PROGRAMS:
        _PROGRAMS[repeat] = _build_program(repeat)
    return _PROGRAMS[repeat]


def _pack_lhsT(blk):
    # [T,T] -> [P,KSUB,T]: out[p,ks,m] = blk[m, ks*128+p]
    return np.ascontiguousarray(blk.T.reshape(KSUB, P, T).transpose(1, 0, 2))


def _pack_rhs(blk):
    # [T,T] -> [P,KSUB,T]: out[p,ks,n] = blk[ks*128+p, n]
    return np.ascontiguousarray(blk.reshape(KSUB, P, T).transpose(1, 0, 2))


def _build_in_maps(A, B):
    tri = np.triu(np.ones((T, T), dtype=np.float32))

    def get_block(mat, bi, bj, pack):
        M = A if mat == "A" else B
        blk = M[bi * T:(bi + 1) * T, bj * T:(bj + 1) * T]
        if bi == bj:
            blk = blk * tri
        return _pack_lhsT(blk) if pack == "L" else _pack_rhs(blk)

    in_maps = []
    for c in range(NCORES):
        sch = _SCHEDULES[c]
        m = {}
        np_in = np.float16 if INPUT_DTYPE == "float16" else np.float32
        for name, specs in [("stat4", sch["stat"]), ("mov8", sch["mov"]),
                            ("shared3", sch["shared"]), ("excl7", sch["excl"])]:
            arr = np.empty((len(specs), P, KSUB, T), dtype=np.float32)
            for t, (mat, bi, bj, pack) in enumerate(specs):
                arr[t] = get_block(mat, bi, bj, pack)
            m[name] = arr.astype(np_in)
        in_maps.append(m)
    return in_maps


def _unpack(results):
    C = np.zeros((N, N), dtype=np.float32)
    for c in range(NCORES):
        out = results[c]["out_stack"].astype(np.float32)  # [NSLOTS, P, KSUB, T]
        for s, (oi, oj, transposed) in enumerate(_SCHEDULES[c]["outs"]):
            # out[s][p, ms, n] = block[ms*128+p, n]
            part = out[s].transpose(1, 0, 2).reshape(T, T)
            if transposed:
                part = part.T
            C[oi * T:(oi + 1) * T, oj * T:(oj + 1) * T] += part
    return C


def kernel(A, B):
    from concourse.bass_utils import run_bass_kernel_spmd

    A = np.asarray(A, dtype=np.float32)
    B = np.asarray(B, dtype=np.float32)
    nc = _get_program()
    in_maps = _build_in_maps(A, B)
    res = run_bass_kernel_spmd(nc, in_maps, list(range(NCORES)))
    return _unpack(res.results)



# revision 10
# speedup vs baseline: 3.9843x; 3.9843x over previous
"""Triangular GEMM C = triu(triu(A) @ triu(B)) for N=4096 fp32 on 8 trn2 cores.

Block decomposition (T=512): C(I,J) = sum_{K=I..J} A(I,K) @ B(K,J) for I<=J,
with diagonal A/B blocks pre-masked triu on host. 120 unit block-matmuls.

Work is packed into a uniform SPMD program (one compiled kernel, per-core
behavior lives entirely in host-packed DRAM stacks):

  per core: 1 "brick" = two depth-4 PSUM K-chains sharing their 4 stationary
  blocks, + 3 groups of singles (sizes 3,2,2) sharing one moving block each.
  = 15 units, 22 input blocks (22 MB), 9 output partials (9 MB).

Transpose trick: C = tA@tB  <=>  C^T = tB^T @ tA^T, so a column-sharing
(B-side) brick/group runs the same program with A/B roles swapped in the
host packing and its output partial transposed on unpack. Uniformity is
preserved; the mode is invisible to the device program.

Host scatter-adds the per-core partials into C. Entries below the diagonal
are exactly zero (every product has a zero factor), matching the reference.
"""

import numpy as np

N = 4096
T = 512  # block size
NB = N // T  # 8
P = 128
KSUB = T // P  # 4
NCORES = 8
NSLOTS = 9

# float16 (e5m10) has the same 11-bit mantissa as float32r (TF32-like), so
# GEMM error is ~1.5e-4 either way (fp32 PSUM accumulation) -- but fp16
# halves input DMA traffic and keeps the fast weight-load path.
INPUT_DTYPE = "float16"  # float16 | float32r | float32
OUT_DTYPE = "float16"    # rounding to fp16 adds ~5e-4 rel err; halves out-DMA
BUFS = dict(stat=4, mov=8, sh=3, ex=7, o=3, psum=8)  # full input residency
LOOP_KW = {}  # extra kwargs for the timing-only For_i repeat loop
PRELOAD = True   # issue all input DMAs up front
OUT_ENGINE = "gpsimd"  # separate DMA queue so stores never block input loads
COPY_ENGINE = "vector"  # psum->sbuf copy engine: any | vector | scalar
N_PREWARM = 40   # PE p-state warmup matmuls (128-col) while first DMAs land

# 8 bricks: two 4-chains sharing the stationary panel.
#   N-mode: row I, stationary A(I,s) for s in S; chains produce C(I,J).
#   T-mode: col J, stationary B(s,J); chains produce C(I,J)^T.
BRICKS = [
    dict(mode="N", line=0, S=[0, 1, 2, 3], outs=[(0, 7), (0, 3)]),
    dict(mode="T", line=7, S=[4, 5, 6, 7], outs=[(0, 7), (4, 7)]),
    dict(mode="N", line=0, S=[0, 1, 2, 3], outs=[(0, 4), (0, 5)]),
    dict(mode="T", line=6, S=[1, 2, 3, 4], outs=[(0, 6), (1, 6)]),
    dict(mode="T", line=7, S=[2, 3, 4, 5], outs=[(1, 7), (2, 7)]),
    dict(mode="N", line=1, S=[1, 2, 3, 4], outs=[(1, 4), (1, 5)]),
    dict(mode="N", line=2, S=[2, 3, 4, 5], outs=[(2, 5), (2, 6)]),
    dict(mode="N", line=3, S=[3, 4, 5, 6], outs=[(3, 6), (3, 7)]),
]

# Singles groups (exact cover of the 56 leftover units; solver output).
# ('A', (i,k), units): shared A(i,k) -> trans mode;
# ('B', (k,j), units): shared B(k,j) -> normal mode. unit = (i,k,j).
GROUPS3 = [
    ("A", (4, 4), [(4, 4, 4), (4, 4, 5), (4, 4, 6)]),
    ("B", (4, 4), [(0, 4, 4), (2, 4, 4), (3, 4, 4)]),
    ("A", (5, 5), [(5, 5, 5), (5, 5, 6), (5, 5, 7)]),
    ("B", (5, 5), [(0, 5, 5), (1, 5, 5), (3, 5, 5)]),
    ("B", (6, 7), [(1, 6, 7), (2, 6, 7), (5, 6, 7)]),
    ("B", (6, 6), [(0, 6, 6), (1, 6, 6), (5, 6, 6)]),
    ("B", (7, 7), [(1, 7, 7), (2, 7, 7), (7, 7, 7)]),
    ("B", (7, 7), [(3, 7, 7), (5, 7, 7), (6, 7, 7)]),
]
GROUPS2 = [
    ("A", (0, 1), [(0, 1, 1), (0, 1, 2)]),
    ("A", (1, 2), [(1, 2, 2), (1, 2, 3)]),
    ("B", (2, 2), [(0, 2, 2), (2, 2, 2)]),
    ("A", (2, 2), [(2, 2, 3), (2, 2, 4)]),
    ("A", (2, 3), [(2, 3, 3), (2, 3, 4)]),
    ("B", (3, 3), [(1, 3, 3), (3, 3, 3)]),
    ("A", (3, 3), [(3, 3, 4), (3, 3, 5)]),
    ("B", (4, 5), [(0, 4, 5), (3, 4, 5)]),
    ("A", (4, 5), [(4, 5, 5), (4, 5, 6)]),
    ("B", (5, 6), [(0, 5, 6), (1, 5, 6)]),
    ("A", (0, 0), [(0, 0, 0), (0, 0, 1)]),
    ("A", (0, 0), [(0, 0, 2), (0, 0, 6)]),
    ("A", (1, 1), [(1, 1, 1), (1, 1, 2)]),
    ("A", (1, 1), [(1, 1, 3), (1, 1, 7)]),
    ("A", (6, 6), [(6, 6, 6), (6, 6, 7)]),
    ("B", (6, 6), [(2, 6, 6), (4, 6, 6)]),
]


def _core_schedule(c):
    """Packing directives for core c.

    Returns dict with block-spec lists; spec = (mat, bi, bj, pack) where
    mat in 'AB', pack 'L' (_pack_lhsT) or 'R' (_pack_rhs); and
    out_specs = [(I, J, transposed)] * 9.
    """
    br = BRICKS[c]
    stat, mov, out_specs = [], [], []
    if br["mode"] == "N":
        i = br["line"]
        stat = [("A", i, s, "L") for s in br["S"]]
        for (oi, oj) in br["outs"]:
            assert oi == i
            mov += [("B", s, oj, "R") for s in br["S"]]
            out_specs.append((oi, oj, False))
    else:
        j = br["line"]
        stat = [("B", s, j, "R") for s in br["S"]]
        for (oi, oj) in br["outs"]:
            assert oj == j
            mov += [("A", oi, s, "L") for s in br["S"]]
            out_specs.append((oi, oj, True))

    shared, excl = [], []
    for grp in [GROUPS3[c], GROUPS2[2 * c], GROUPS2[2 * c + 1]]:
        gmode, key, units = grp
        if gmode == "A":
            gi, gk = key
            shared.append(("A", gi, gk, "L"))
            for (ui, uk, uj) in units:
                assert (ui, uk) == key
                excl.append(("B", uk, uj, "R"))
                out_specs.append((ui, uj, True))
        else:
            gk, gj = key
            shared.append(("B", gk, gj, "R"))
            for (ui, uk, uj) in units:
                assert (uk, uj) == key
                excl.append(("A", ui, uk, "L"))
                out_specs.append((ui, uj, False))
    assert len(stat) == 4 and len(mov) == 8
    assert len(shared) == 3 and len(excl) == 7 and len(out_specs) == 9
    return dict(stat=stat, mov=mov, shared=shared, excl=excl, outs=out_specs)


_SCHEDULES = [_core_schedule(c) for c in range(NCORES)]


def _check_cover():
    seen = set()
    for c in range(NCORES):
        br = BRICKS[c]
        for (oi, oj) in br["outs"]:
            for s in br["S"]:
                u = (oi, s, oj) if br["mode"] == "N" else (oi, s, oj)
                assert oi <= s <= oj, (c, u)
                assert u not in seen, u
                seen.add(u)
        for grp in [GROUPS3[c], GROUPS2[2 * c], GROUPS2[2 * c + 1]]:
            for u in grp[2]:
                i, k, j = u
                assert i <= k <= j, u
                assert u not in seen, u
                seen.add(u)
    want = {(i, k, j) for i in range(NB) for k in range(i, NB)
            for j in range(k, NB)}
    assert seen == want, (len(seen), len(want))


_check_cover()

_PROGRAMS = {}


def _build_program(repeat=1):
    import contextlib
    import concourse.bacc as bacc
    import concourse.mybir as mybir
    from concourse.tile import TileContext

    dt_in = getattr(mybir.dt, INPUT_DTYPE)
    nc = bacc.Bacc("TRN2", target_bir_lowering=False, debug=False,
                   num_devices=NCORES)
    stat_in = nc.dram_tensor("stat4", [4, P, KSUB, T], dt_in,
                             kind="ExternalInput")
    mov_in = nc.dram_tensor("mov8", [8, P, KSUB, T], dt_in,
                            kind="ExternalInput")
    sh_in = nc.dram_tensor("shared3", [3, P, KSUB, T], dt_in,
                           kind="ExternalInput")
    ex_in = nc.dram_tensor("excl7", [7, P, KSUB, T], dt_in,
                           kind="ExternalInput")
    dt_out = getattr(mybir.dt, OUT_DTYPE)
    # [s, p, ms, n]: per-partition-contiguous rows -> full-rate DMA
    c_out = nc.dram_tensor("out_stack", [NSLOTS, P, KSUB, T],
                           dt_out, kind="ExternalOutput")

    f32 = mybir.dt.float32

    with TileContext(nc) as tc:
        with (
            tc.tile_pool(name="stat_pool", bufs=BUFS["stat"]) as stat_pool,
            tc.tile_pool(name="mov_pool", bufs=BUFS["mov"]) as mov_pool,
            tc.tile_pool(name="sh_pool", bufs=BUFS["sh"]) as sh_pool,
            tc.tile_pool(name="ex_pool", bufs=BUFS["ex"]) as ex_pool,
            tc.tile_pool(name="o_pool", bufs=BUFS["o"]) as o_pool,
            tc.tile_pool(name="warm_pool", bufs=1) as warm_pool,
            tc.tile_pool(name="psum", bufs=BUFS["psum"], space="PSUM") as psum_pool,
        ):
            out_eng = getattr(nc, OUT_ENGINE)
            copy_eng = getattr(nc, COPY_ENGINE)

            # PE p-state warmup: the tensor engine ramps 0.65->1.2->2.4 GHz
            # with sustained use; real matmuls gated on DMA arrival start
            # cold and stay slow. Run a stream of tiny matmuls on a memset
            # tile so the PE is warm when the first input tile lands.
            if N_PREWARM:
                w_t = warm_pool.tile([P, 2 * P], dt_in, tag="warm", name="warm")
                nc.vector.memset(w_t[:, :], 0.0)
                pw_ps = psum_pool.tile([P, T], mybir.dt.float32, tag="ps",
                                       name="pwps")
                for i in range(N_PREWARM):
                    nc.tensor.matmul(pw_ps[:, 0:P], w_t[:, 0:P], w_t[:, P:2 * P],
                                     start=True, stop=True)

            loop_ctx = (tc.For_i(0, repeat, 1, **LOOP_KW) if repeat > 1
                        else contextlib.nullcontext())
            with loop_ctx:
                def store(psums, slot):
                    o_t = o_pool.tile([P, KSUB, T], dt_out, tag="o",
                                      name=f"o_{slot}")
                    for ms in range(KSUB):
                        if COPY_ENGINE == "scalar":
                            copy_eng.copy(o_t[:, ms, :], psums[ms][:, :])
                        else:
                            copy_eng.tensor_copy(o_t[:, ms, :], psums[ms][:, :])
                    out_eng.dma_start(out=c_out[slot], in_=o_t)

                def load(pool, tag, name, src):
                    t_ = pool.tile([P, KSUB, T], dt_in, tag=tag, name=name)
                    nc.sync.dma_start(out=t_, in_=src)
                    return t_

                # issue in exact consumption order: (stat_u, mov_u) pairs for
                # chain 0, then chain 1 movs, then each singles group's
                # shared tile followed by its exclusives. Pairing stats with
                # movs brings the first matmul's deps in ~2 DMAs instead of 5.
                stat_t, mov_t = [None] * 4, [None] * 8
                for u in range(4):
                    stat_t[u] = load(stat_pool, "st", f"st_{u}", stat_in[u])
                    if PRELOAD:
                        mov_t[u] = load(mov_pool, "mv", f"mv_{u}", mov_in[u])
                if PRELOAD:
                    for u in range(4, 8):
                        mov_t[u] = load(mov_pool, "mv", f"mv_{u}", mov_in[u])
                    sh_t_all, ex_t_all = [], []
                    e = 0
                    for g, gsize in enumerate([3, 2, 2]):
                        sh_t_all.append(load(sh_pool, "sh", f"sh_{g}",
                                             sh_in[g]))
                        for _ in range(gsize):
                            ex_t_all.append(load(ex_pool, "ex", f"ex_{e}",
                                                 ex_in[e]))
                            e += 1

                for ch in range(2):
                    psums = [psum_pool.tile([P, T], f32, tag="ps",
                                            name=f"ps_b{ch}_{m}")
                             for m in range(KSUB)]
                    for u in range(4):
                        m_t = (mov_t[ch * 4 + u] if PRELOAD else
                               load(mov_pool, "mv", f"mv_{ch}_{u}",
                                    mov_in[ch * 4 + u]))
                        # ms-outer: psum[ms]'s chain finishes 4 matmuls before
                        # psum[ms+1]'s, so its copy overlaps remaining compute
                        for ms in range(KSUB):
                            for ks in range(KSUB):
                                nc.tensor.matmul(
                                    psums[ms][:, :],
                                    stat_t[u][:, ks, ms * P:(ms + 1) * P],
                                    m_t[:, ks, :],
                                    start=(u == 0 and ks == 0),
                                    stop=(u == 3 and ks == KSUB - 1),
                                )
                    store(psums, ch)

                # singles groups (3, 2, 2)
                slot = 2
                e_idx = 0
                for g, gsize in enumerate([3, 2, 2]):
                    sh_t = (sh_t_all[g] if PRELOAD else
                            load(sh_pool, "sh", f"sh_{g}", sh_in[g]))
                    for q in range(gsize):
                        e_t = (ex_t_all[e_idx] if PRELOAD else
                               load(ex_pool, "ex", f"ex_{g}_{q}", ex_in[e_idx]))
                        psums = [psum_pool.tile([P, T], f32, tag="ps",
                                                name=f"ps_s{slot}_{m}")
                                 for m in range(KSUB)]
                        for ms in range(KSUB):
                            for ks in range(KSUB):
                                nc.tensor.matmul(
                                    psums[ms][:, :],
                                    e_t[:, ks, ms * P:(ms + 1) * P],
                                    sh_t[:, ks, :],
                                    start=(ks == 0),
                                    stop=(ks == KSUB - 1),
                                )
                        store(psums, slot)
                        slot += 1
                        e_idx += 1
    nc.finalize()
    return nc


def _get_program(repeat=1):
    if repeat not in _PROGRAMS:
        _PROGRAMS[repeat] = _build_program(repeat)
    return _PROGRAMS[repeat]


def _pack_lhsT(blk):
    # [T,T] -> [P,KSUB,T]: out[p,ks,m] = blk[m, ks*128+p]
    return np.ascontiguousarray(blk.T.reshape(KSUB, P, T).transpose(1, 0, 2))


def _pack_rhs(blk):
    # [T,T] -> [P,KSUB,T]: out[p,ks,n] = blk[ks*128+p, n]
    return np.ascontiguousarray(blk.reshape(KSUB, P, T).transpose(1, 0, 2))


def _build_in_maps(A, B):
    tri = np.triu(np.ones((T, T), dtype=np.float32))

    def get_block(mat, bi, bj, pack):
        M = A if mat == "A" else B
        blk = M[bi * T:(bi + 1) * T, bj * T:(bj + 1) * T]
        if bi == bj:
            blk = blk * tri
        return _pack_lhsT(blk) if pack == "L" else _pack_rhs(blk)

    in_maps = []
    for c in range(NCORES):
        sch = _SCHEDULES[c]
        m = {}
        np_in = np.float16 if INPUT_DTYPE == "float16" else np.float32
        for name, specs in [("stat4", sch["stat"]), ("mov8", sch["mov"]),
                            ("shared3", sch["shared"]), ("excl7", sch["excl"])]:
            arr = np.empty((len(specs), P, KSUB, T), dtype=np.float32)
            for t, (mat, bi, bj, pack) in enumerate(specs):
                arr[t] = get_block(mat, bi, bj, pack)
            m[name] = arr.astype(np_in)
        in_maps.append(m)
    return in_maps


def _unpack(results):
    C = np.zeros((N, N), dtype=np.float32)
    for c in range(NCORES):
        out = results[c]["out_stack"].astype(np.float32)  # [NSLOTS, P, KSUB, T]
        for s, (oi, oj, transposed) in enumerate(_SCHEDULES[c]["outs"]):
            # out[s][p, ms, n] = block[ms*128+p, n]
            part = out[s].transpose(1, 0, 2).reshape(T, T)
            if transposed:
                part = part.T
            C[oi * T:(oi + 1) * T, oj * T:(oj + 1) * T] += part
    return C


def kernel(A, B):
    from concourse.bass_utils import run_bass_kernel_spmd

    A = np.asarray(A, dtype=np.float32)
    B = np.asarray(B, dtype=np.float32)
    nc = _get_program()
    in_maps = _build_in_maps(A, B)
    res = run_bass_kernel_spmd(nc, in_maps, list(range(NCORES)))
    return _unpack(res.results)



# revision 11
# speedup vs baseline: 4.9887x; 1.2521x over previous
"""Triangular GEMM C = triu(triu(A) @ triu(B)), N=4096 fp32, 8 trn2 cores.

Tri-skip design: C(I,J) = sum_{K=I..J} A(I,K)B(K,J); units (i,k,j) with a
triangular (diagonal) factor skip its all-zero 128x128 sub-chunks:
  Rs: tri block as stationary -> 10 of 16 weight chunks (10 MMs x 512)
  Rm: tri block as moving     -> 16 MMs with column runs [128ks, 512)
  X (i,i,i): both              -> 10 MMs with column runs
Dense D: 16 MMs x 512. Per core: 7 D + 7 R + 1 X = 95744 columns
(vs 122880 dense) -> PE floor 39.9us warm.

Uniform SPMD skeleton (same instruction stream on all 8 cores):
  brick row r: C4=[Rs 3D] + C3=[Rs 2D] sharing tri1 + 3 panel stats
  pair-s: 2x Rs sharing tri1;  pair-m: 2x Rm sharing a compact moving
  single: Rs (tri3);  dmid: 2x D sharing a stat;  X: tri1 + compact moving
10 output slots (fp16), 22 input tiles (18 dense + 4 tri-compact).

Cores 4-7 run the same schedules on the flipped problem
A' = flip(B^T), B' = flip(A^T)  (flip = reverse both axes), which is again
upper-triangular; their outputs map back via block (bi,bj) -> (7-bj,7-bi),
content M -> M[::-1,::-1].T. Plans cover all 120 units exactly (see
plan_f.py).
"""

import numpy as np

N = 4096
T = 512
NB = 8
P = 128
KSUB = 4
NCORES = 8
NSLOTS = 10
NDENSE = 18
NTRI = 4
TRIW = 10 * P  # compact tri tile width (10 chunks of 128)

INPUT_DTYPE = "float16"
OUT_DTYPE = "float16"
N_PREWARM = 36

# (ks, ms) chunk order for tri-stationary compacts (ks >= ms)
TRI_CHUNKS = [(0, 0), (1, 0), (1, 1), (2, 0), (2, 1), (2, 2),
              (3, 0), (3, 1), (3, 2), (3, 3)]
MOFF = [0, 512, 896, 1152]   # compact moving run offsets per ks
MLEN = [512, 384, 256, 128]  # run lengths (= 512 - 128*ks)

# planner output (plan_f.py): base plans; core c uses PLANS[c % 4],
# cores 4-7 on the flipped problem.
PLANS = [
    dict(r=0, J1=7, panel=(1, 2, 3), J2=3, c3ks=(1, 2),
         dmid=((0, 1, 2), (0, 1, 6)), pair=(1, 2), single=(0, 4),
         rm=[(2, 7, 7), (1, 7, 7)], kappa=7),
    dict(r=1, J1=7, panel=(2, 3, 4), J2=6, c3ks=(2, 3),
         dmid=((0, 2, 5), (0, 2, 6)), pair=(2, 3), single=(2, 5),
         rm=[(3, 6, 6), (2, 6, 6)], kappa=6),
    dict(r=2, J1=7, panel=(3, 4, 6), J2=6, c3ks=(3, 4),
         dmid=((1, 2, 3), (1, 2, 5)), pair=(3, 4), single=(4, 7),
         rm=[(2, 3, 3), (1, 3, 3)], kappa=3),
    dict(r=3, J1=7, panel=(4, 5, 6), J2=6, c3ks=(4, 5),
         dmid=((2, 3, 4), (2, 3, 5)), pair=(4, 5), single=(6, 7),
         rm=[(1, 2, 2), (0, 2, 2)], kappa=2),
]


def _schedule(plan):
    """Tile specs + out specs for one base plan (core space).

    dense tiles (18, DMA order interleaved with tri tiles; tri indices in
    the 4-wide tri stack): returns
      dense: list of ('A'|'B', bi, bj, 'L'|'R') in stack order
      tri:   list of ('A'|'B', bi, bj, 'SC'|'MC') (stat/mov compact)
      dma_order: list of ('d'|'t', idx) issue order
      outs:  10 x (bi, bj, isX)
    """
    r, J1, J2 = plan["r"], plan["J1"], plan["J2"]
    panel, c3ks = list(plan["panel"]), list(plan["c3ks"])
    s3k = [k for k in panel if k not in c3ks][0]
    pan = c3ks + [s3k]  # s1, s2 = C3's stats; s3 = C4-only
    (ia, kap, _), (ib, _, _) = plan["rm"]
    sg, js = plan["single"]
    (di, dk1, dj1), (_, dk2, dj2) = plan["dmid"]
    assert plan["dmid"][0][:2] == plan["dmid"][1][:2]
    jc, jd = plan["pair"]

    dense = [
        ("B", r, J1, "R"),        # 0 mrs1 (C4 Rs moving)
        ("A", r, pan[0], "L"),    # 1 s1
        ("B", pan[0], J1, "R"),   # 2 m11
        ("A", r, pan[1], "L"),    # 3 s2
        ("B", pan[1], J1, "R"),   # 4 m12
        ("A", r, pan[2], "L"),    # 5 s3
        ("B", pan[2], J1, "R"),   # 6 m13
        ("B", r, J2, "R"),        # 7 mrs2 (C3 Rs moving)
        ("B", pan[0], J2, "R"),   # 8 m21
        ("B", pan[1], J2, "R"),   # 9 m22
        ("B", r, jc, "R"),        # 10 pa
        ("B", r, jd, "R"),        # 11 pb
        ("A", ia, kap, "L"),      # 12 ra
        ("A", ib, kap, "L"),      # 13 rb
        ("B", sg, js, "R"),       # 14 msg
        ("A", di, dk1, "L"),      # 15 dstat
        ("B", dk1, dj1, "R"),     # 16 da
        ("B", dk2, dj2, "R"),     # 17 db
    ]
    assert dk1 == dk2
    tri = [
        ("A", r, r, "SC"),        # 0 tri1
        ("B", kap, kap, "MC"),    # 1 mc
        ("A", sg, sg, "SC"),      # 2 tri3
        ("B", r, r, "MC"),        # 3 movx
    ]
    # issue order = consumption order of the group sequence
    # C4, C3, X, pair-m, pair-s, single, dmid
    dma_order = [("d", 1), ("d", 2), ("t", 0), ("d", 0), ("d", 3),
                 ("d", 4), ("d", 5), ("d", 6), ("d", 7), ("d", 8),
                 ("d", 9), ("t", 3), ("t", 1), ("d", 12), ("d", 13),
                 ("d", 10), ("d", 11), ("t", 2), ("d", 14), ("d", 15),
                 ("d", 16), ("d", 17)]
    outs = [(r, J1, False), (r, J2, False), (r, jc, False), (r, jd, False),
            (ia, kap, False), (ib, kap, False), (sg, js, False),
            (di, dj1, False), (di, dj2, False), (r, r, True)]
    return dict(dense=dense, tri=tri, dma_order=dma_order, outs=outs)


_SCHEDS = [_schedule(p) for p in PLANS]


def _check_cover():
    def phi(u):
        i, k, j = u
        return (NB - 1 - j, NB - 1 - k, NB - 1 - i)

    flat = []
    for p in PLANS:
        r, J1, J2 = p["r"], p["J1"], p["J2"]
        units = ([(r, r, J1)] + [(r, k, J1) for k in p["panel"]] +
                 [(r, r, J2)] + [(r, k, J2) for k in p["c3ks"]] +
                 [(r, r, p["pair"][0]), (r, r, p["pair"][1])] +
                 [tuple(u) for u in p["rm"]] +
                 [(p["single"][0], p["single"][0], p["single"][1])] +
                 [tuple(u) for u in p["dmid"]] + [(r, r, r)])
        assert len(units) == 15
        flat += units
    cover = set(flat) | {phi(u) for u in flat}
    want = {(i, k, j) for i in range(NB) for k in range(i, NB)
            for j in range(k, NB)}
    assert len(flat) == 60 and len(set(flat)) == 60 and cover == want


_check_cover()

_PROGRAMS = {}


def _build_program(repeat=1):
    import contextlib
    import concourse.bacc as bacc
    import concourse.mybir as mybir
    from concourse.tile import TileContext

    dt_in = getattr(mybir.dt, INPUT_DTYPE)
    dt_out = getattr(mybir.dt, OUT_DTYPE)
    f32 = mybir.dt.float32
    nc = bacc.Bacc("TRN2", target_bir_lowering=False, debug=False,
                   num_devices=NCORES)
    dense_in = nc.dram_tensor("dense18", [NDENSE, P, KSUB, T], dt_in,
                              kind="ExternalInput")
    tri_in = nc.dram_tensor("tri4", [NTRI, P, TRIW], dt_in,
                            kind="ExternalInput")
    c_out = nc.dram_tensor("out_stack", [NSLOTS, P, KSUB, T], dt_out,
                           kind="ExternalOutput")

    with TileContext(nc) as tc:
        with (
            tc.tile_pool(name="dense_pool", bufs=NDENSE) as dense_pool,
            tc.tile_pool(name="tri_pool", bufs=NTRI) as tri_pool,
            tc.tile_pool(name="o_pool", bufs=NSLOTS) as o_pool,
            tc.tile_pool(name="warm_pool", bufs=1) as warm_pool,
            tc.tile_pool(name="psum", bufs=8, space="PSUM") as psum_pool,
        ):
            # PE p-state warmup on a memset tile (see kernel.py)
            if N_PREWARM:
                w_t = warm_pool.tile([P, 2 * P], dt_in, tag="warm",
                                     name="warm")
                nc.vector.memset(w_t[:, :], 0.0)
                pw_ps = psum_pool.tile([P, T], f32, tag="ps", name="pwps")
                for i in range(N_PREWARM):
                    nc.tensor.matmul(pw_ps[:, 0:P], w_t[:, 0:P],
                                     w_t[:, P:2 * P], start=True, stop=True)

            loop_ctx = (tc.For_i(0, repeat, 1) if repeat > 1
                        else contextlib.nullcontext())
            with loop_ctx:
                dt_, tt_ = [None] * NDENSE, [None] * NTRI
                for (kind, idx) in _SCHEDS[0]["dma_order"]:
                    if kind == "d":
                        dt_[idx] = dense_pool.tile([P, KSUB, T], dt_in,
                                                   tag="dn", name=f"dn{idx}")
                        nc.sync.dma_start(out=dt_[idx], in_=dense_in[idx])
                    else:
                        tt_[idx] = tri_pool.tile([P, TRIW], dt_in,
                                                 tag="tr", name=f"tr{idx}")
                        nc.sync.dma_start(out=tt_[idx], in_=tri_in[idx])

                def store(psums, slot, eng=0, last=False):
                    # split the 4 copies across DVE and ACT: one engine's
                    # queue can't keep up with back-to-back small groups
                    # (4x658ns copies vs 2.1us compute per group)
                    o_t = o_pool.tile([P, KSUB, T], dt_out, tag="o",
                                      name=f"o_{slot}")
                    for ms in range(KSUB):
                        if (ms + eng) % 2 == 0:
                            nc.vector.tensor_copy(o_t[:, ms, :],
                                                  psums[ms][:, :])
                        else:
                            nc.scalar.copy(o_t[:, ms, :], psums[ms][:, :])
                        if last:
                            # tail: ship each ms-slice as its copy lands on
                            # the HWDGE queue (skips Pool Q7 launch) so the
                            # final DMA carries only 128KB
                            nc.sync.dma_start(out=c_out[slot, :, ms, :],
                                              in_=o_t[:, ms, :])
                    if not last:
                        nc.gpsimd.dma_start(out=c_out[slot], in_=o_t)

                def new_psums(nm):
                    return [psum_pool.tile([P, T], f32, tag="ps",
                                           name=f"ps_{nm}_{m}")
                            for m in range(KSUB)]

                def mm_rs(psums, tri_t, mov_t, start, stop):
                    # tri-stationary: 10 chunks, full 512-col streams
                    for q, (ks, ms) in enumerate(TRI_CHUNKS):
                        nc.tensor.matmul(
                            psums[ms][:, :], tri_t[:, q * P:(q + 1) * P],
                            mov_t[:, ks, :],
                            start=start and (ks == ms),
                            stop=stop and (ks == 3))

                def mm_d(psums, stat_t, mov_t, start, stop):
                    # dense unit, ms-outer for copy stagger
                    for ms in range(KSUB):
                        for ks in range(KSUB):
                            nc.tensor.matmul(
                                psums[ms][:, :],
                                stat_t[:, ks, ms * P:(ms + 1) * P],
                                mov_t[:, ks, :],
                                start=start and (ks == 0),
                                stop=stop and (ks == 3))

                def mm_rm(psums, stat_t, mc_t):
                    # tri-moving: column runs [128ks, 512)
                    for ms in range(KSUB):
                        for ks in range(KSUB):
                            nc.tensor.matmul(
                                psums[ms][:, ks * P:T],
                                stat_t[:, ks, ms * P:(ms + 1) * P],
                                mc_t[:, MOFF[ks]:MOFF[ks] + MLEN[ks]],
                                start=(ks == 0), stop=(ks == 3))

                def mm_x(psums, tri_t, mx_t):
                    # both triangular: 10 chunks, column runs
                    for q, (ks, ms) in enumerate(TRI_CHUNKS):
                        nc.tensor.matmul(
                            psums[ms][:, ks * P:T],
                            tri_t[:, q * P:(q + 1) * P],
                            mx_t[:, MOFF[ks]:MOFF[ks] + MLEN[ks]],
                            start=(ks == ms), stop=(ks == 3))

                # execution order: C4, C3, X, pair-m, pair-s, single, dmid;
                # the long staggered dmid groups last -> short tail
                # g0: C4 = Rs + 3 D   (tri1; movings 0,2,4,6; stats 1,3,5)
                ps = new_psums("c4")
                mm_d(ps, dt_[1], dt_[2], True, False)
                mm_rs(ps, tt_[0], dt_[0], False, False)
                mm_d(ps, dt_[3], dt_[4], False, False)
                mm_d(ps, dt_[5], dt_[6], False, True)
                store(ps, 0, 0)
                # g1: C3 = Rs + 2 D   (tri1; movings 7,8,9; stats 1,3)
                ps = new_psums("c3")
                mm_rs(ps, tt_[0], dt_[7], True, False)
                mm_d(ps, dt_[1], dt_[8], False, False)
                mm_d(ps, dt_[3], dt_[9], False, True)
                store(ps, 1, 1)
                # g9: X
                ps = new_psums("x")
                mm_x(ps, tt_[0], tt_[3])
                store(ps, 9, 0)
                # g4, g5: pair-m (Rm)
                for g, si, e in ((4, 12, 1), (5, 13, 0)):
                    ps = new_psums(f"m{g}")
                    mm_rm(ps, dt_[si], tt_[1])
                    store(ps, g, e)
                # g2, g3: pair-s
                for g, mi, e in ((2, 10, 1), (3, 11, 0)):
                    ps = new_psums(f"p{g}")
                    mm_rs(ps, tt_[0], dt_[mi], True, True)
                    store(ps, g, e)
                # g6: single Rs (tri3)
                ps = new_psums("sg")
                mm_rs(ps, tt_[2], dt_[14], True, True)
                store(ps, 6, 1)
                # g7, g8: dmid D singles (shared stat 15)
                for g, mi, e in ((7, 16, 0), (8, 17, 1)):
                    ps = new_psums(f"d{g}")
                    mm_d(ps, dt_[15], dt_[mi], True, True)
                    store(ps, g, e, last=(g == 8))
    nc.finalize()
    return nc


def _get_program(repeat=1):
    if repeat not in _PROGRAMS:
        _PROGRAMS[repeat] = _build_program(repeat)
    return _PROGRAMS[repeat]


def _pack_lhsT(blk):
    # [T,T] -> [P,KSUB,T]: out[p,ks,m] = blk[m, ks*128+p]
    return np.ascontiguousarray(blk.T.reshape(KSUB, P, T).transpose(1, 0, 2))


def _pack_rhs(blk):
    # [T,T] -> [P,KSUB,T]: out[p,ks,n] = blk[ks*128+p, n]
    return np.ascontiguousarray(blk.reshape(KSUB, P, T).transpose(1, 0, 2))


def _pack_sc(blk):
    # tri stat compact: chunks q=(ks,ms) of the L-pack, [P, 10*128]
    lp = _pack_lhsT(blk)
    out = np.empty((P, TRIW), dtype=blk.dtype)
    for q, (ks, ms) in enumerate(TRI_CHUNKS):
        out[:, q * P:(q + 1) * P] = lp[:, ks, ms * P:(ms + 1) * P]
    return out


def _pack_mc(blk):
    # tri moving compact: runs [128ks, 512) of the R-pack, [P, 1280]
    rp = _pack_rhs(blk)
    out = np.empty((P, TRIW), dtype=blk.dtype)
    for ks in range(KSUB):
        out[:, MOFF[ks]:MOFF[ks] + MLEN[ks]] = rp[:, ks, ks * P:T]
    return out


_PACK = {"L": _pack_lhsT, "R": _pack_rhs, "SC": _pack_sc, "MC": _pack_mc}


def _build_in_maps(A, B):
    tri = np.triu(np.ones((T, T), dtype=np.float32))
    np_in = np.float16 if INPUT_DTYPE == "float16" else np.float32
    in_maps = []
    for c in range(NCORES):
        if c < 4:
            Ac, Bc = A, B
        else:
            Ac = B.T[::-1, ::-1]
            Bc = A.T[::-1, ::-1]
        sch = _SCHEDS[c % 4]

        def get(mat, bi, bj, pack):
            M = Ac if mat == "A" else Bc
            blk = M[bi * T:(bi + 1) * T, bj * T:(bj + 1) * T]
            if bi == bj:
                blk = blk * tri
            return _PACK[pack](blk.astype(np.float32))

        d = np.empty((NDENSE, P, KSUB, T), dtype=np.float32)
        for t, (mat, bi, bj, pack) in enumerate(sch["dense"]):
            d[t] = get(mat, bi, bj, pack)
        tr = np.empty((NTRI, P, TRIW), dtype=np.float32)
        for t, (mat, bi, bj, pack) in enumerate(sch["tri"]):
            tr[t] = get(mat, bi, bj, pack)
        in_maps.append({"dense18": d.astype(np_in),
                        "tri4": tr.astype(np_in)})
    return in_maps


def _unpack(results):
    tri = np.triu(np.ones((T, T), dtype=np.float32))
    C = np.zeros((N, N), dtype=np.float32)
    for c in range(NCORES):
        out = results[c]["out_stack"].astype(np.float32)
        for s, (bi, bj, isX) in enumerate(_SCHEDS[c % 4]["outs"]):
            part = out[s].transpose(1, 0, 2).reshape(T, T)
            if isX:
                part = part * tri  # kill never-written psum regions
            if c >= 4:
                bi, bj, part = NB - 1 - bj, NB - 1 - bi, part[::-1, ::-1].T
            C[bi * T:(bi + 1) * T, bj * T:(bj + 1) * T] += part
    return C


def kernel(A, B):
    from concourse.bass_utils import run_bass_kernel_spmd

    A = np.asarray(A, dtype=np.float32)
    B = np.asarray(B, dtype=np.float32)
    nc = _get_program()
    in_maps = _build_in_maps(A, B)
    res = run_bass_kernel_spmd(nc, in_maps, list(range(NCORES)))
    return _unpack(res.results)


# revision 12
# speedup vs baseline: 5.0249x; 1.0073x over previous
"""Triangular GEMM C = triu(triu(A) @ triu(B)), N=4096 fp32, 8 trn2 cores.

Tri-skip design: C(I,J) = sum_{K=I..J} A(I,K)B(K,J); units (i,k,j) with a
triangular (diagonal) factor skip its all-zero 128x128 sub-chunks:
  Rs: tri block as stationary -> 10 of 16 weight chunks (10 MMs x 512)
  Rm: tri block as moving     -> 16 MMs with column runs [128ks, 512)
  X (i,i,i): both              -> 10 MMs with column runs
Dense D: 16 MMs x 512. Per core: 7 D + 7 R + 1 X = 95744 columns
(vs 122880 dense) -> PE floor 39.9us warm.

Uniform SPMD skeleton (same instruction stream on all 8 cores):
  brick row r: C4=[Rs 3D] + C3=[Rs 2D] sharing tri1 + 3 panel stats
  pair-s: 2x Rs sharing tri1;  pair-m: 2x Rm sharing a compact moving
  single: Rs (tri3);  dmid: 2x D sharing a stat;  X: tri1 + compact moving
10 output slots (fp16), 22 input tiles (18 dense + 4 tri-compact).

Cores 4-7 run the same schedules on the flipped problem
A' = flip(B^T), B' = flip(A^T)  (flip = reverse both axes), which is again
upper-triangular; their outputs map back via block (bi,bj) -> (7-bj,7-bi),
content M -> M[::-1,::-1].T. Plans cover all 120 units exactly (see
plan_f.py).
"""

import numpy as np

N = 4096
T = 512
NB = 8
P = 128
KSUB = 4
NCORES = 8
NSLOTS = 10
NDENSE = 18
NTRI = 4
TRIW = 10 * P  # compact tri tile width (10 chunks of 128)

INPUT_DTYPE = "float16"
OUT_DTYPE = "float16"
N_PREWARM = 30

# (ks, ms) chunk order for tri-stationary compacts (ks >= ms)
TRI_CHUNKS = [(0, 0), (1, 0), (1, 1), (2, 0), (2, 1), (2, 2),
              (3, 0), (3, 1), (3, 2), (3, 3)]
MOFF = [0, 512, 896, 1152]   # compact moving run offsets per ks
MLEN = [512, 384, 256, 128]  # run lengths (= 512 - 128*ks)

# planner output (plan_f.py): base plans; core c uses PLANS[c % 4],
# cores 4-7 on the flipped problem.
PLANS = [
    dict(r=0, J1=7, panel=(1, 2, 3), J2=3, c3ks=(1, 2),
         dmid=((0, 1, 2), (0, 1, 6)), pair=(1, 2), single=(0, 4),
         rm=[(2, 7, 7), (1, 7, 7)], kappa=7),
    dict(r=1, J1=7, panel=(2, 3, 4), J2=6, c3ks=(2, 3),
         dmid=((0, 2, 5), (0, 2, 6)), pair=(2, 3), single=(2, 5),
         rm=[(3, 6, 6), (2, 6, 6)], kappa=6),
    dict(r=2, J1=7, panel=(3, 4, 6), J2=6, c3ks=(3, 4),
         dmid=((1, 2, 3), (1, 2, 5)), pair=(3, 4), single=(4, 7),
         rm=[(2, 3, 3), (1, 3, 3)], kappa=3),
    dict(r=3, J1=7, panel=(4, 5, 6), J2=6, c3ks=(4, 5),
         dmid=((2, 3, 4), (2, 3, 5)), pair=(4, 5), single=(6, 7),
         rm=[(1, 2, 2), (0, 2, 2)], kappa=2),
]


def _schedule(plan):
    """Tile specs + out specs for one base plan (core space).

    dense tiles (18, DMA order interleaved with tri tiles; tri indices in
    the 4-wide tri stack): returns
      dense: list of ('A'|'B', bi, bj, 'L'|'R') in stack order
      tri:   list of ('A'|'B', bi, bj, 'SC'|'MC') (stat/mov compact)
      dma_order: list of ('d'|'t', idx) issue order
      outs:  10 x (bi, bj, isX)
    """
    r, J1, J2 = plan["r"], plan["J1"], plan["J2"]
    panel, c3ks = list(plan["panel"]), list(plan["c3ks"])
    s3k = [k for k in panel if k not in c3ks][0]
    pan = c3ks + [s3k]  # s1, s2 = C3's stats; s3 = C4-only
    (ia, kap, _), (ib, _, _) = plan["rm"]
    sg, js = plan["single"]
    (di, dk1, dj1), (_, dk2, dj2) = plan["dmid"]
    assert plan["dmid"][0][:2] == plan["dmid"][1][:2]
    jc, jd = plan["pair"]

    dense = [
        ("B", r, J1, "R"),        # 0 mrs1 (C4 Rs moving)
        ("A", r, pan[0], "L"),    # 1 s1
        ("B", pan[0], J1, "R"),   # 2 m11
        ("A", r, pan[1], "L"),    # 3 s2
        ("B", pan[1], J1, "R"),   # 4 m12
        ("A", r, pan[2], "L"),    # 5 s3
        ("B", pan[2], J1, "R"),   # 6 m13
        ("B", r, J2, "R"),        # 7 mrs2 (C3 Rs moving)
        ("B", pan[0], J2, "R"),   # 8 m21
        ("B", pan[1], J2, "R"),   # 9 m22
        ("B", r, jc, "R"),        # 10 pa
        ("B", r, jd, "R"),        # 11 pb
        ("A", ia, kap, "L"),      # 12 ra
        ("A", ib, kap, "L"),      # 13 rb
        ("B", sg, js, "R"),       # 14 msg
        ("A", di, dk1, "L"),      # 15 dstat
        ("B", dk1, dj1, "R"),     # 16 da
        ("B", dk2, dj2, "R"),     # 17 db
    ]
    assert dk1 == dk2
    tri = [
        ("A", r, r, "SC"),        # 0 tri1
        ("B", kap, kap, "MC"),    # 1 mc
        ("A", sg, sg, "SC"),      # 2 tri3
        ("B", r, r, "MC"),        # 3 movx
    ]
    # issue order = consumption order of the group sequence
    # C4, C3, X, pair-m, pair-s, single, dmid
    dma_order = [("d", 1), ("d", 2), ("t", 0), ("d", 0), ("d", 3),
                 ("d", 4), ("d", 5), ("d", 6), ("d", 7), ("d", 8),
                 ("d", 9), ("t", 3), ("t", 1), ("d", 12), ("d", 13),
                 ("d", 10), ("d", 11), ("t", 2), ("d", 14), ("d", 15),
                 ("d", 16), ("d", 17)]
    outs = [(r, J1, False), (r, J2, False), (r, jc, False), (r, jd, False),
            (ia, kap, False), (ib, kap, False), (sg, js, False),
            (di, dj1, False), (di, dj2, False), (r, r, True)]
    return dict(dense=dense, tri=tri, dma_order=dma_order, outs=outs)


_SCHEDS = [_schedule(p) for p in PLANS]


def _check_cover():
    def phi(u):
        i, k, j = u
        return (NB - 1 - j, NB - 1 - k, NB - 1 - i)

    flat = []
    for p in PLANS:
        r, J1, J2 = p["r"], p["J1"], p["J2"]
        units = ([(r, r, J1)] + [(r, k, J1) for k in p["panel"]] +
                 [(r, r, J2)] + [(r, k, J2) for k in p["c3ks"]] +
                 [(r, r, p["pair"][0]), (r, r, p["pair"][1])] +
                 [tuple(u) for u in p["rm"]] +
                 [(p["single"][0], p["single"][0], p["single"][1])] +
                 [tuple(u) for u in p["dmid"]] + [(r, r, r)])
        assert len(units) == 15
        flat += units
    cover = set(flat) | {phi(u) for u in flat}
    want = {(i, k, j) for i in range(NB) for k in range(i, NB)
            for j in range(k, NB)}
    assert len(flat) == 60 and len(set(flat)) == 60 and cover == want


_check_cover()

_PROGRAMS = {}


def _build_program(repeat=1):
    import contextlib
    import concourse.bacc as bacc
    import concourse.mybir as mybir
    from concourse.tile import TileContext

    dt_in = getattr(mybir.dt, INPUT_DTYPE)
    dt_out = getattr(mybir.dt, OUT_DTYPE)
    f32 = mybir.dt.float32
    nc = bacc.Bacc("TRN2", target_bir_lowering=False, debug=False,
                   num_devices=NCORES)
    dense_in = nc.dram_tensor("dense18", [NDENSE, P, KSUB, T], dt_in,
                              kind="ExternalInput")
    tri_in = nc.dram_tensor("tri4", [NTRI, P, TRIW], dt_in,
                            kind="ExternalInput")
    c_out = nc.dram_tensor("out_stack", [NSLOTS, P, KSUB, T], dt_out,
                           kind="ExternalOutput")

    with TileContext(nc) as tc:
        with (
            tc.tile_pool(name="dense_pool", bufs=NDENSE) as dense_pool,
            tc.tile_pool(name="tri_pool", bufs=NTRI) as tri_pool,
            tc.tile_pool(name="o_pool", bufs=NSLOTS) as o_pool,
            tc.tile_pool(name="warm_pool", bufs=1) as warm_pool,
            tc.tile_pool(name="psum", bufs=8, space="PSUM") as psum_pool,
        ):
            # PE p-state warmup on a memset tile (see kernel.py)
            if N_PREWARM:
                w_t = warm_pool.tile([P, 2 * P], dt_in, tag="warm",
                                     name="warm")
                nc.vector.memset(w_t[:, :], 0.0)
                pw_ps = psum_pool.tile([P, T], f32, tag="ps", name="pwps")
                for i in range(N_PREWARM):
                    nc.tensor.matmul(pw_ps[:, 0:P], w_t[:, 0:P],
                                     w_t[:, P:2 * P], start=True, stop=True)

            loop_ctx = (tc.For_i(0, repeat, 1) if repeat > 1
                        else contextlib.nullcontext())
            with loop_ctx:
                dt_, tt_ = [None] * NDENSE, [None] * NTRI
                first = True
                for (kind, idx) in _SCHEDS[0]["dma_order"]:
                    if kind == "d":
                        if idx == 2:
                            continue  # loaded together with idx 1 below
                        dt_[idx] = dense_pool.tile([P, KSUB, T], dt_in,
                                                   tag="dn", name=f"dn{idx}")
                        if first and idx == 1:
                            # halve the first stat/moving loads, interleaved
                            # (s1h1, m11h1, s1h2, m11h2): the lead dense unit
                            # runs ks-outer and starts on the first halves
                            dt_[2] = dense_pool.tile([P, KSUB, T], dt_in,
                                                     tag="dn", name="dn2")
                            for h in range(2):
                                for t_i in (1, 2):
                                    nc.sync.dma_start(
                                        out=dt_[t_i][:, 2 * h:2 * h + 2, :],
                                        in_=dense_in[t_i][:, 2 * h:2 * h + 2, :])
                            first = False
                        else:
                            nc.sync.dma_start(out=dt_[idx],
                                              in_=dense_in[idx])
                    else:
                        tt_[idx] = tri_pool.tile([P, TRIW], dt_in,
                                                 tag="tr", name=f"tr{idx}")
                        nc.sync.dma_start(out=tt_[idx], in_=tri_in[idx])

                def store(psums, slot, eng=0, last=False):
                    # split the 4 copies across DVE and ACT: one engine's
                    # queue can't keep up with back-to-back small groups
                    # (4x658ns copies vs 2.1us compute per group)
                    o_t = o_pool.tile([P, KSUB, T], dt_out, tag="o",
                                      name=f"o_{slot}")
                    for ms in range(KSUB):
                        if (ms + eng) % 2 == 0:
                            nc.vector.tensor_copy(o_t[:, ms, :],
                                                  psums[ms][:, :])
                        else:
                            nc.scalar.copy(o_t[:, ms, :], psums[ms][:, :])
                        if last:
                            # tail: ship each ms-slice as its copy lands on
                            # the HWDGE queue (skips Pool Q7 launch) so the
                            # final DMA carries only 128KB
                            nc.sync.dma_start(out=c_out[slot, :, ms, :],
                                              in_=o_t[:, ms, :])
                    if not last:
                        nc.gpsimd.dma_start(out=c_out[slot], in_=o_t)

                def new_psums(nm):
                    return [psum_pool.tile([P, T], f32, tag="ps",
                                           name=f"ps_{nm}_{m}")
                            for m in range(KSUB)]

                def mm_rs(psums, tri_t, mov_t, start, stop):
                    # tri-stationary: 10 chunks, full 512-col streams
                    for q, (ks, ms) in enumerate(TRI_CHUNKS):
                        nc.tensor.matmul(
                            psums[ms][:, :], tri_t[:, q * P:(q + 1) * P],
                            mov_t[:, ks, :],
                            start=start and (ks == ms),
                            stop=stop and (ks == 3))

                def mm_d(psums, stat_t, mov_t, start, stop):
                    # dense unit, ms-outer for copy stagger
                    for ms in range(KSUB):
                        for ks in range(KSUB):
                            nc.tensor.matmul(
                                psums[ms][:, :],
                                stat_t[:, ks, ms * P:(ms + 1) * P],
                                mov_t[:, ks, :],
                                start=start and (ks == 0),
                                stop=stop and (ks == 3))

                def mm_rm(psums, stat_t, mc_t):
                    # tri-moving: column runs [128ks, 512)
                    for ms in range(KSUB):
                        for ks in range(KSUB):
                            nc.tensor.matmul(
                                psums[ms][:, ks * P:T],
                                stat_t[:, ks, ms * P:(ms + 1) * P],
                                mc_t[:, MOFF[ks]:MOFF[ks] + MLEN[ks]],
                                start=(ks == 0), stop=(ks == 3))

                def mm_x(psums, tri_t, mx_t):
                    # both triangular: 10 chunks, column runs
                    for q, (ks, ms) in enumerate(TRI_CHUNKS):
                        nc.tensor.matmul(
                            psums[ms][:, ks * P:T],
                            tri_t[:, q * P:(q + 1) * P],
                            mx_t[:, MOFF[ks]:MOFF[ks] + MLEN[ks]],
                            start=(ks == ms), stop=(ks == 3))

                # execution order: C4, C3, X, pair-m, pair-s, single, dmid;
                # the long staggered dmid groups last -> short tail
                # g0: C4 = Rs + 3 D   (tri1; movings 0,2,4,6; stats 1,3,5)
                ps = new_psums("c4")
                for ks in range(KSUB):   # ks-outer: start on half tiles
                    for ms in range(KSUB):
                        nc.tensor.matmul(
                            ps[ms][:, :], dt_[1][:, ks, ms * P:(ms + 1) * P],
                            dt_[2][:, ks, :], start=(ks == 0), stop=False)
                mm_rs(ps, tt_[0], dt_[0], False, False)
                mm_d(ps, dt_[3], dt_[4], False, False)
                mm_d(ps, dt_[5], dt_[6], False, True)
                store(ps, 0, 0)
                # g1: C3 = Rs + 2 D   (tri1; movings 7,8,9; stats 1,3)
                ps = new_psums("c3")
                mm_rs(ps, tt_[0], dt_[7], True, False)
                mm_d(ps, dt_[1], dt_[8], False, False)
                mm_d(ps, dt_[3], dt_[9], False, True)
                store(ps, 1, 1)
                # g9: X
                ps = new_psums("x")
                mm_x(ps, tt_[0], tt_[3])
                store(ps, 9, 0)
                # g4, g5: pair-m (Rm)
                for g, si, e in ((4, 12, 1), (5, 13, 0)):
                    ps = new_psums(f"m{g}")
                    mm_rm(ps, dt_[si], tt_[1])
                    store(ps, g, e)
                # g2, g3: pair-s
                for g, mi, e in ((2, 10, 1), (3, 11, 0)):
                    ps = new_psums(f"p{g}")
                    mm_rs(ps, tt_[0], dt_[mi], True, True)
                    store(ps, g, e)
                # g6: single Rs (tri3)
                ps = new_psums("sg")
                mm_rs(ps, tt_[2], dt_[14], True, True)
                store(ps, 6, 1)
                # g7, g8: dmid D singles (shared stat 15)
                for g, mi, e in ((7, 16, 0), (8, 17, 1)):
                    ps = new_psums(f"d{g}")
                    mm_d(ps, dt_[15], dt_[mi], True, True)
                    store(ps, g, e, last=(g == 8))
    nc.finalize()
    return nc


def _get_program(repeat=1):
    if repeat not in _PROGRAMS:
        _PROGRAMS[repeat] = _build_program(repeat)
    return _PROGRAMS[repeat]


def _pack_lhsT(blk):
    # [T,T] -> [P,KSUB,T]: out[p,ks,m] = blk[m, ks*128+p]
    return np.ascontiguousarray(blk.T.reshape(KSUB, P, T).transpose(1, 0, 2))


def _pack_rhs(blk):
    # [T,T] -> [P,KSUB,T]: out[p,ks,n] = blk[ks*128+p, n]
    return np.ascontiguousarray(blk.reshape(KSUB, P, T).transpose(1, 0, 2))


def _pack_sc(blk):
    # tri stat compact: chunks q=(ks,ms) of the L-pack, [P, 10*128]
    lp = _pack_lhsT(blk)
    out = np.empty((P, TRIW), dtype=blk.dtype)
    for q, (ks, ms) in enumerate(TRI_CHUNKS):
        out[:, q * P:(q + 1) * P] = lp[:, ks, ms * P:(ms + 1) * P]
    return out


def _pack_mc(blk):
    # tri moving compact: runs [128ks, 512) of the R-pack, [P, 1280]
    rp = _pack_rhs(blk)
    out = np.empty((P, TRIW), dtype=blk.dtype)
    for ks in range(KSUB):
        out[:, MOFF[ks]:MOFF[ks] + MLEN[ks]] = rp[:, ks, ks * P:T]
    return out


_PACK = {"L": _pack_lhsT, "R": _pack_rhs, "SC": _pack_sc, "MC": _pack_mc}


def _build_in_maps(A, B):
    tri = np.triu(np.ones((T, T), dtype=np.float32))
    np_in = np.float16 if INPUT_DTYPE == "float16" else np.float32
    in_maps = []
    for c in range(NCORES):
        if c < 4:
            Ac, Bc = A, B
        else:
            Ac = B.T[::-1, ::-1]
            Bc = A.T[::-1, ::-1]
        sch = _SCHEDS[c % 4]

        def get(mat, bi, bj, pack):
            M = Ac if mat == "A" else Bc
            blk = M[bi * T:(bi + 1) * T, bj * T:(bj + 1) * T]
            if bi == bj:
                blk = blk * tri
            return _PACK[pack](blk.astype(np.float32))

        d = np.empty((NDENSE, P, KSUB, T), dtype=np.float32)
        for t, (mat, bi, bj, pack) in enumerate(sch["dense"]):
            d[t] = get(mat, bi, bj, pack)
        tr = np.empty((NTRI, P, TRIW), dtype=np.float32)
        for t, (mat, bi, bj, pack) in enumerate(sch["tri"]):
            tr[t] = get(mat, bi, bj, pack)
        in_maps.append({"dense18": d.astype(np_in),
                        "tri4": tr.astype(np_in)})
    return in_maps


def _unpack(results):
    tri = np.triu(np.ones((T, T), dtype=np.float32))
    C = np.zeros((N, N), dtype=np.float32)
    for c in range(NCORES):
        out = results[c]["out_stack"].astype(np.float32)
        for s, (bi, bj, isX) in enumerate(_SCHEDS[c % 4]["outs"]):
            part = out[s].transpose(1, 0, 2).reshape(T, T)
            if isX:
                part = part * tri  # kill never-written psum regions
            if c >= 4:
                bi, bj, part = NB - 1 - bj, NB - 1 - bi, part[::-1, ::-1].T
            C[bi * T:(bi + 1) * T, bj * T:(bj + 1) * T] += part
    return C


def kernel(A, B):
    from concourse.bass_utils import run_bass_kernel_spmd

    A = np.asarray(A, dtype=np.float32)
    B = np.asarray(B, dtype=np.float32)
    nc = _get_program()
    in_maps = _build_in_maps(A, B)
    res = run_bass_kernel_spmd(nc, in_maps, list(range(NCORES)))
    return _unpack(res.results)


# revision 14
# speedup vs baseline: 5.1178x; 1.0185x over previous
"""Triangular GEMM C = triu(triu(A) @ triu(B)), N=4096 fp32, 8 trn2 cores.

Tri-skip design: C(I,J) = sum_{K=I..J} A(I,K)B(K,J); units (i,k,j) with a
triangular (diagonal) factor skip its all-zero 128x128 sub-chunks:
  Rs: tri block as stationary -> 10 of 16 weight chunks (10 MMs x 512)
  Rm: tri block as moving     -> 16 MMs with column runs [128ks, 512)
  X (i,i,i): both              -> 10 MMs with column runs
Dense D: 16 MMs x 512. Per core: 7 D + 7 R + 1 X = 95744 columns
(vs 122880 dense) -> PE floor 39.9us warm.

Uniform SPMD skeleton (same instruction stream on all 8 cores):
  brick row r: C4=[Rs 3D] + C3=[Rs 2D] sharing tri1 + 3 panel stats
  pair-s: 2x Rs sharing tri1;  pair-m: 2x Rm sharing a compact moving
  single: Rs (tri3);  dmid: 2x D sharing a stat;  X: tri1 + compact moving
10 output slots (fp16), 22 input tiles (18 dense + 4 tri-compact).

Cores 4-7 run the same schedules on the flipped problem
A' = flip(B^T), B' = flip(A^T)  (flip = reverse both axes), which is again
upper-triangular; their outputs map back via block (bi,bj) -> (7-bj,7-bi),
content M -> M[::-1,::-1].T. Plans cover all 120 units exactly (see
plan_f.py).
"""

import numpy as np

N = 4096
T = 512
NB = 8
P = 128
KSUB = 4
NCORES = 8
NSLOTS = 10
NDENSE = 18
NTRI = 4
TRIW = 10 * P  # compact tri tile width (10 chunks of 128)

INPUT_DTYPE = "float16"
OUT_DTYPE = "float16"
N_PREWARM = 30

# (ks, ms) chunk order for tri-stationary compacts (ks >= ms)
TRI_CHUNKS = [(0, 0), (1, 0), (1, 1), (2, 0), (2, 1), (2, 2),
              (3, 0), (3, 1), (3, 2), (3, 3)]
MOFF = [0, 512, 896, 1152]   # compact moving run offsets per ks
MLEN = [512, 384, 256, 128]  # run lengths (= 512 - 128*ks)

# planner output (plan_f.py): base plans; core c uses PLANS[c % 4],
# cores 4-7 on the flipped problem.
PLANS = [
    dict(r=0, J1=7, panel=(1, 2, 3), J2=3, c3ks=(1, 2),
         dmid=((0, 1, 2), (0, 1, 6)), pair=(1, 2), single=(0, 4),
         rm=[(2, 7, 7), (1, 7, 7)], kappa=7),
    dict(r=1, J1=7, panel=(2, 3, 4), J2=6, c3ks=(2, 3),
         dmid=((0, 2, 5), (0, 2, 6)), pair=(2, 3), single=(2, 5),
         rm=[(3, 6, 6), (2, 6, 6)], kappa=6),
    dict(r=2, J1=7, panel=(3, 4, 6), J2=6, c3ks=(3, 4),
         dmid=((1, 2, 3), (1, 2, 5)), pair=(3, 4), single=(4, 7),
         rm=[(2, 3, 3), (1, 3, 3)], kappa=3),
    dict(r=3, J1=7, panel=(4, 5, 6), J2=6, c3ks=(4, 5),
         dmid=((2, 3, 4), (2, 3, 5)), pair=(4, 5), single=(6, 7),
         rm=[(1, 2, 2), (0, 2, 2)], kappa=2),
]


def _schedule(plan):
    """Tile specs + out specs for one base plan (core space).

    dense tiles (18, DMA order interleaved with tri tiles; tri indices in
    the 4-wide tri stack): returns
      dense: list of ('A'|'B', bi, bj, 'L'|'R') in stack order
      tri:   list of ('A'|'B', bi, bj, 'SC'|'MC') (stat/mov compact)
      dma_order: list of ('d'|'t', idx) issue order
      outs:  10 x (bi, bj, isX)
    """
    r, J1, J2 = plan["r"], plan["J1"], plan["J2"]
    panel, c3ks = list(plan["panel"]), list(plan["c3ks"])
    s3k = [k for k in panel if k not in c3ks][0]
    pan = c3ks + [s3k]  # s1, s2 = C3's stats; s3 = C4-only
    (ia, kap, _), (ib, _, _) = plan["rm"]
    sg, js = plan["single"]
    (di, dk1, dj1), (_, dk2, dj2) = plan["dmid"]
    assert plan["dmid"][0][:2] == plan["dmid"][1][:2]
    jc, jd = plan["pair"]

    dense = [
        ("B", r, J1, "R"),        # 0 mrs1 (C4 Rs moving)
        ("A", r, pan[0], "L"),    # 1 s1
        ("B", pan[0], J1, "R"),   # 2 m11
        ("A", r, pan[1], "L"),    # 3 s2
        ("B", pan[1], J1, "R"),   # 4 m12
        ("A", r, pan[2], "L"),    # 5 s3
        ("B", pan[2], J1, "R"),   # 6 m13
        ("B", r, J2, "R"),        # 7 mrs2 (C3 Rs moving)
        ("B", pan[0], J2, "R"),   # 8 m21
        ("B", pan[1], J2, "R"),   # 9 m22
        ("B", r, jc, "R"),        # 10 pa
        ("B", r, jd, "R"),        # 11 pb
        ("A", ia, kap, "L"),      # 12 ra
        ("A", ib, kap, "L"),      # 13 rb
        ("B", sg, js, "R"),       # 14 msg
        ("A", di, dk1, "L"),      # 15 dstat
        ("B", dk1, dj1, "R"),     # 16 da
        ("B", dk2, dj2, "R"),     # 17 db
    ]
    assert dk1 == dk2
    tri = [
        ("A", r, r, "SC"),        # 0 tri1
        ("B", kap, kap, "MC"),    # 1 mc
        ("A", sg, sg, "SC"),      # 2 tri3
        ("B", r, r, "MC"),        # 3 movx
    ]
    # issue order = consumption order of the group sequence
    # C4, C3, X, pair-m, pair-s, single, dmid
    dma_order = [("d", 1), ("d", 2), ("t", 0), ("d", 0), ("d", 3),
                 ("d", 4), ("d", 5), ("d", 6), ("d", 7), ("d", 8),
                 ("d", 9), ("t", 3), ("t", 1), ("d", 12), ("d", 13),
                 ("d", 10), ("d", 11), ("t", 2), ("d", 14), ("d", 15),
                 ("d", 16), ("d", 17)]
    outs = [(r, J1, False), (r, J2, False), (r, jc, False), (r, jd, False),
            (ia, kap, False), (ib, kap, False), (sg, js, False),
            (di, dj1, False), (di, dj2, False), (r, r, True)]
    return dict(dense=dense, tri=tri, dma_order=dma_order, outs=outs)


_SCHEDS = [_schedule(p) for p in PLANS]


def _check_cover():
    def phi(u):
        i, k, j = u
        return (NB - 1 - j, NB - 1 - k, NB - 1 - i)

    flat = []
    for p in PLANS:
        r, J1, J2 = p["r"], p["J1"], p["J2"]
        units = ([(r, r, J1)] + [(r, k, J1) for k in p["panel"]] +
                 [(r, r, J2)] + [(r, k, J2) for k in p["c3ks"]] +
                 [(r, r, p["pair"][0]), (r, r, p["pair"][1])] +
                 [tuple(u) for u in p["rm"]] +
                 [(p["single"][0], p["single"][0], p["single"][1])] +
                 [tuple(u) for u in p["dmid"]] + [(r, r, r)])
        assert len(units) == 15
        flat += units
    cover = set(flat) | {phi(u) for u in flat}
    want = {(i, k, j) for i in range(NB) for k in range(i, NB)
            for j in range(k, NB)}
    assert len(flat) == 60 and len(set(flat)) == 60 and cover == want


_check_cover()

_PROGRAMS = {}


def _build_program(repeat=1):
    import contextlib
    import concourse.bacc as bacc
    import concourse.mybir as mybir
    from concourse.tile import TileContext

    dt_in = getattr(mybir.dt, INPUT_DTYPE)
    dt_out = getattr(mybir.dt, OUT_DTYPE)
    f32 = mybir.dt.float32
    nc = bacc.Bacc("TRN2", target_bir_lowering=False, debug=False,
                   num_devices=NCORES)
    dense_in = nc.dram_tensor("dense18", [NDENSE, P, KSUB, T], dt_in,
                              kind="ExternalInput")
    tri_in = nc.dram_tensor("tri4", [NTRI, P, TRIW], dt_in,
                            kind="ExternalInput")
    c_out = nc.dram_tensor("out_stack", [NSLOTS, P, KSUB, T], dt_out,
                           kind="ExternalOutput")

    with TileContext(nc) as tc:
        with (
            tc.tile_pool(name="dense_pool", bufs=NDENSE) as dense_pool,
            tc.tile_pool(name="tri_pool", bufs=NTRI) as tri_pool,
            tc.tile_pool(name="o_pool", bufs=NSLOTS) as o_pool,
            tc.tile_pool(name="warm_pool", bufs=1) as warm_pool,
            tc.tile_pool(name="psum", bufs=8, space="PSUM") as psum_pool,
        ):
            # PE p-state warmup on a memset tile (see kernel.py)
            if N_PREWARM:
                w_t = warm_pool.tile([P, 2 * P], dt_in, tag="warm",
                                     name="warm")
                nc.vector.memset(w_t[:, :], 0.0)
                pw_ps = psum_pool.tile([P, T], f32, tag="ps", name="pwps")
                for i in range(N_PREWARM):
                    nc.tensor.matmul(pw_ps[:, 0:P], w_t[:, 0:P],
                                     w_t[:, P:2 * P], start=True, stop=True)

            loop_ctx = (tc.For_i(0, repeat, 1) if repeat > 1
                        else contextlib.nullcontext())
            with loop_ctx:
                dt_, tt_ = [None] * NDENSE, [None] * NTRI
                first = True
                for (kind, idx) in _SCHEDS[0]["dma_order"]:
                    if kind == "d":
                        if idx in (2, 4, 6):
                            continue  # loaded together with idx 1 / 3 / 5
                        dt_[idx] = dense_pool.tile([P, KSUB, T], dt_in,
                                                   tag="dn", name=f"dn{idx}")
                        if idx in (3, 5):
                            # half-split s2/m12 and s3/m13 too; D2/D3 run
                            # ks-outer so they start on half tiles
                            dt_[idx + 1] = dense_pool.tile(
                                [P, KSUB, T], dt_in, tag="dn",
                                name=f"dn{idx + 1}")
                            for h in range(2):
                                for t_i in (idx, idx + 1):
                                    nc.sync.dma_start(
                                        out=dt_[t_i][:, 2 * h:2 * h + 2, :],
                                        in_=dense_in[t_i][:, 2 * h:2 * h + 2, :])
                            continue
                        if first and idx == 1:
                            # halve the first stat/moving loads, interleaved
                            # (s1h1, m11h1, s1h2, m11h2): the lead dense unit
                            # runs ks-outer and starts on the first halves
                            dt_[2] = dense_pool.tile([P, KSUB, T], dt_in,
                                                     tag="dn", name="dn2")
                            for h in range(2):
                                for t_i in (1, 2):
                                    nc.sync.dma_start(
                                        out=dt_[t_i][:, 2 * h:2 * h + 2, :],
                                        in_=dense_in[t_i][:, 2 * h:2 * h + 2, :])
                            first = False
                        else:
                            nc.sync.dma_start(out=dt_[idx],
                                              in_=dense_in[idx])
                    else:
                        tt_[idx] = tri_pool.tile([P, TRIW], dt_in,
                                                 tag="tr", name=f"tr{idx}")
                        nc.sync.dma_start(out=tt_[idx], in_=tri_in[idx])

                def store(psums, slot, eng=0, last=False):
                    # split the 4 copies across DVE and ACT: one engine's
                    # queue can't keep up with back-to-back small groups
                    # (4x658ns copies vs 2.1us compute per group)
                    o_t = o_pool.tile([P, KSUB, T], dt_out, tag="o",
                                      name=f"o_{slot}")
                    for ms in range(KSUB):
                        if (ms + eng) % 2 == 0:
                            nc.vector.tensor_copy(o_t[:, ms, :],
                                                  psums[ms][:, :])
                        else:
                            nc.scalar.copy(o_t[:, ms, :], psums[ms][:, :])
                        if last:
                            # tail: ship each ms-slice as its copy lands on
                            # the HWDGE queue (skips Pool Q7 launch) so the
                            # final DMA carries only 128KB
                            nc.sync.dma_start(out=c_out[slot, :, ms, :],
                                              in_=o_t[:, ms, :])
                    if not last:
                        nc.gpsimd.dma_start(out=c_out[slot], in_=o_t)

                def new_psums(nm):
                    return [psum_pool.tile([P, T], f32, tag="ps",
                                           name=f"ps_{nm}_{m}")
                            for m in range(KSUB)]

                def mm_rs(psums, tri_t, mov_t, start, stop):
                    # tri-stationary: 10 chunks, full 512-col streams
                    for q, (ks, ms) in enumerate(TRI_CHUNKS):
                        nc.tensor.matmul(
                            psums[ms][:, :], tri_t[:, q * P:(q + 1) * P],
                            mov_t[:, ks, :],
                            start=start and (ks == ms),
                            stop=stop and (ks == 3))

                def mm_d(psums, stat_t, mov_t, start, stop):
                    # dense unit, ms-outer for copy stagger
                    for ms in range(KSUB):
                        for ks in range(KSUB):
                            nc.tensor.matmul(
                                psums[ms][:, :],
                                stat_t[:, ks, ms * P:(ms + 1) * P],
                                mov_t[:, ks, :],
                                start=start and (ks == 0),
                                stop=stop and (ks == 3))

                def mm_rm(psums, stat_t, mc_t):
                    # tri-moving: column runs [128ks, 512)
                    for ms in range(KSUB):
                        for ks in range(KSUB):
                            nc.tensor.matmul(
                                psums[ms][:, ks * P:T],
                                stat_t[:, ks, ms * P:(ms + 1) * P],
                                mc_t[:, MOFF[ks]:MOFF[ks] + MLEN[ks]],
                                start=(ks == 0), stop=(ks == 3))

                def mm_x(psums, tri_t, mx_t):
                    # both triangular: 10 chunks, column runs
                    for q, (ks, ms) in enumerate(TRI_CHUNKS):
                        nc.tensor.matmul(
                            psums[ms][:, ks * P:T],
                            tri_t[:, q * P:(q + 1) * P],
                            mx_t[:, MOFF[ks]:MOFF[ks] + MLEN[ks]],
                            start=(ks == ms), stop=(ks == 3))

                # execution order: C4, C3, X, pair-m, pair-s, single, dmid;
                # the long staggered dmid groups last -> short tail
                # g0: C4 = Rs + 3 D   (tri1; movings 0,2,4,6; stats 1,3,5)
                ps = new_psums("c4")
                for ks in range(KSUB):   # ks-outer: start on half tiles
                    for ms in range(KSUB):
                        nc.tensor.matmul(
                            ps[ms][:, :], dt_[1][:, ks, ms * P:(ms + 1) * P],
                            dt_[2][:, ks, :], start=(ks == 0), stop=False)
                mm_rs(ps, tt_[0], dt_[0], False, False)
                for si in (3, 5):   # D2/D3 ks-outer: start on half tiles
                    for ks in range(KSUB):
                        for ms in range(KSUB):
                            nc.tensor.matmul(
                                ps[ms][:, :],
                                dt_[si][:, ks, ms * P:(ms + 1) * P],
                                dt_[si + 1][:, ks, :], start=False,
                                stop=(si == 5 and ks == KSUB - 1))
                store(ps, 0, 0)
                # g1: C3 = Rs + 2 D   (tri1; movings 7,8,9; stats 1,3)
                ps = new_psums("c3")
                mm_rs(ps, tt_[0], dt_[7], True, False)
                mm_d(ps, dt_[1], dt_[8], False, False)
                mm_d(ps, dt_[3], dt_[9], False, True)
                store(ps, 1, 1)
                # g9: X
                ps = new_psums("x")
                mm_x(ps, tt_[0], tt_[3])
                store(ps, 9, 0)
                # g4, g5: pair-m (Rm)
                for g, si, e in ((4, 12, 1), (5, 13, 0)):
                    ps = new_psums(f"m{g}")
                    mm_rm(ps, dt_[si], tt_[1])
                    store(ps, g, e)
                # g2, g3: pair-s
                for g, mi, e in ((2, 10, 1), (3, 11, 0)):
                    ps = new_psums(f"p{g}")
                    mm_rs(ps, tt_[0], dt_[mi], True, True)
                    store(ps, g, e)
                # g6: single Rs (tri3)
                ps = new_psums("sg")
                mm_rs(ps, tt_[2], dt_[14], True, True)
                store(ps, 6, 1)
                # g7, g8: dmid D singles (shared stat 15)
                for g, mi, e in ((7, 16, 0), (8, 17, 1)):
                    ps = new_psums(f"d{g}")
                    mm_d(ps, dt_[15], dt_[mi], True, True)
                    store(ps, g, e, last=(g == 8))
    nc.finalize()
    return nc


def _get_program(repeat=1):
    if repeat not in _PROGRAMS:
        _PROGRAMS[repeat] = _build_program(repeat)
    return _PROGRAMS[repeat]


def _pack_lhsT(blk):
    # [T,T] -> [P,KSUB,T]: out[p,ks,m] = blk[m, ks*128+p]
    return np.ascontiguousarray(blk.T.reshape(KSUB, P, T).transpose(1, 0, 2))


def _pack_rhs(blk):
    # [T,T] -> [P,KSUB,T]: out[p,ks,n] = blk[ks*128+p, n]
    return np.ascontiguousarray(blk.reshape(KSUB, P, T).transpose(1, 0, 2))


def _pack_sc(blk):
    # tri stat compact: chunks q=(ks,ms) of the L-pack, [P, 10*128]
    lp = _pack_lhsT(blk)
    out = np.empty((P, TRIW), dtype=blk.dtype)
    for q, (ks, ms) in enumerate(TRI_CHUNKS):
        out[:, q * P:(q + 1) * P] = lp[:, ks, ms * P:(ms + 1) * P]
    return out


def _pack_mc(blk):
    # tri moving compact: runs [128ks, 512) of the R-pack, [P, 1280]
    rp = _pack_rhs(blk)
    out = np.empty((P, TRIW), dtype=blk.dtype)
    for ks in range(KSUB):
        out[:, MOFF[ks]:MOFF[ks] + MLEN[ks]] = rp[:, ks, ks * P:T]
    return out


_PACK = {"L": _pack_lhsT, "R": _pack_rhs, "SC": _pack_sc, "MC": _pack_mc}


def _build_in_maps(A, B):
    tri = np.triu(np.ones((T, T), dtype=np.float32))
    np_in = np.float16 if INPUT_DTYPE == "float16" else np.float32
    in_maps = []
    for c in range(NCORES):
        if c < 4:
            Ac, Bc = A, B
        else:
            Ac = B.T[::-1, ::-1]
            Bc = A.T[::-1, ::-1]
        sch = _SCHEDS[c % 4]

        def get(mat, bi, bj, pack):
            M = Ac if mat == "A" else Bc
            blk = M[bi * T:(bi + 1) * T, bj * T:(bj + 1) * T]
            if bi == bj:
                blk = blk * tri
            return _PACK[pack](blk.astype(np.float32))

        d = np.empty((NDENSE, P, KSUB, T), dtype=np.float32)
        for t, (mat, bi, bj, pack) in enumerate(sch["dense"]):
            d[t] = get(mat, bi, bj, pack)
        tr = np.empty((NTRI, P, TRIW), dtype=np.float32)
        for t, (mat, bi, bj, pack) in enumerate(sch["tri"]):
            tr[t] = get(mat, bi, bj, pack)
        in_maps.append({"dense18": d.astype(np_in),
                        "tri4": tr.astype(np_in)})
    return in_maps


def _unpack(results):
    tri = np.triu(np.ones((T, T), dtype=np.float32))
    C = np.zeros((N, N), dtype=np.float32)
    for c in range(NCORES):
        out = results[c]["out_stack"].astype(np.float32)
        for s, (bi, bj, isX) in enumerate(_SCHEDS[c % 4]["outs"]):
            part = out[s].transpose(1, 0, 2).reshape(T, T)
            if isX:
                part = part * tri  # kill never-written psum regions
            if c >= 4:
                bi, bj, part = NB - 1 - bj, NB - 1 - bi, part[::-1, ::-1].T
            C[bi * T:(bi + 1) * T, bj * T:(bj + 1) * T] += part
    return C


def kernel(A, B):
    from concourse.bass_utils import run_bass_kernel_spmd

    A = np.asarray(A, dtype=np.float32)
    B = np.asarray(B, dtype=np.float32)
    nc = _get_program()
    in_maps = _build_in_maps(A, B)
    res = run_bass_kernel_spmd(nc, in_maps, list(range(NCORES)))
    return _unpack(res.results)
